# revision 5
# baseline (speedup 1.0000x reference)
"""Trainium2 Bass kernel for nn_ConexaoRegional.

Reference computation:
    out[b, n, d, s] = sum_r xd[b, n, r] * wd[n, d, s, r]
where
    xd[b, (i,j), r] = x[b, 0, 4i+r, 4j+r]     (patch diagonal)
    wd[n, d, s, r]  = pesos[n, d, s, r, r]    (weight diagonal)

Shapes: x [64,1,128,128] f32, pesos [1024,16,32,4,4] f32,
        out [64,1024,16,32] f32 (128 MiB -> memory-bound).

Strategy: shard the region axis (n) across 8 cores (128 regions each,
contiguous patch-row blocks). Host packs, per core:
  - a block-diagonal stationary operand "xbd" [8, 64*128]: for each pair
    of regions (2p, 2p+1), an [8, 128] tile whose rows are (c*4+r) and
    cols (c*64+b), holding xd values on the two diagonal blocks.
  - a moving operand "wt" [8, 64*512]: for each pair an [8, 512] tile
    with wt[c*4+r, ds] = wd[2p+c, ds, r].
On device each pair is one K=8 matmul -> PSUM [128, 512] = out for the
2 regions x 64 batches x 512 (d,s); copy PSUM->SBUF (alternating
Vector/Scalar engines) and stream 2 MiB chunks to DRAM with 2 KiB
contiguous runs. All FLOPs (the einsum contraction) happen on device.
"""

import numpy as np

B = 64
R = 4
GH = GW = 32
N = GH * GW            # 1024 regions
D, S = 16, 32
DS = D * S             # 512
NCORES = 8
NPC = N // NCORES      # 128 regions per core
PAIRS = NPC // 2       # 64 matmuls per core
GCH = 4                # pairs per output store chunk
NCHUNK = PAIRS // GCH  # 16 chunks -> 2 MiB per store DMA

_NC_CACHE = {}


def _build_bass():
    if "nc" in _NC_CACHE:
        return _NC_CACHE["nc"]
    from contextlib import ExitStack

    import concourse.bacc as bacc
    import concourse.mybir as mybir
    import concourse.tile as tile

    f32 = mybir.dt.float32
    nc = bacc.Bacc()  # Bacc (not raw Bass): its compile passes split multi-sem
    # waits and move matmul waits to ldweights, which TRN2 codegen requires.
    xbd = nc.declare_dram_parameter("xbd", [8, PAIRS * 128], f32, isOutput=False)
    wt = nc.declare_dram_parameter("wt", [8, PAIRS * DS], f32, isOutput=False)
    out = nc.declare_dram_parameter("out", [B, NPC, DS], f32, isOutput=True)

    with ExitStack() as ctx:
        tc = ctx.enter_context(tile.TileContext(nc))
        const = ctx.enter_context(tc.tile_pool(name="const", bufs=1))
        wpool = ctx.enter_context(tc.tile_pool(name="wtp", bufs=3))
        pspool = ctx.enter_context(tc.tile_pool(name="ps", bufs=6, space="PSUM"))
        opool = ctx.enter_context(tc.tile_pool(name="ostage", bufs=3))

        xbd_sb = const.tile([8, PAIRS * 128], f32)
        nc.sync.dma_start(xbd_sb[:], xbd[:])

        for g in range(NCHUNK):
            wt_sb = wpool.tile([8, GCH * DS], f32)
            nc.sync.dma_start(wt_sb[:], wt[:, g * GCH * DS:(g + 1) * GCH * DS])
            ostage = opool.tile([128, GCH * DS], f32)
            for j in range(GCH):
                p = g * GCH + j
                ps = pspool.tile([128, DS], f32)
                nc.tensor.matmul(
                    ps[:],
                    lhsT=xbd_sb[:, p * 128:(p + 1) * 128],
                    rhs=wt_sb[:, j * DS:(j + 1) * DS],
                    start=True,
                    stop=True,
                )
                dst = ostage[:, j * DS:(j + 1) * DS]
                if j % 2 == 0:
                    nc.vector.tensor_copy(dst, ps[:])
                else:
                    nc.scalar.copy(dst, ps[:])
            # out[b, 8g + 2p' + c, ds]; src rows are (c*64+b), free (p', ds).
            # DMA APs max 3 dims -> one store per parity c: dst [b, p', ds].
            for c in range(2):
                dst_ap = out[:, 8 * g + c:8 * (g + 1):2, :]
                nc.sync.dma_start(dst_ap, ostage[c * 64:(c + 1) * 64, :])

    nc.compile()  # Bacc passes: reg alloc, wait splitting, ldweights fixup
    _NC_CACHE["nc"] = nc
    return nc


def _pack_inputs(x, pesos):
    x = np.ascontiguousarray(np.asarray(x), dtype=np.float32)
    pesos = np.ascontiguousarray(np.asarray(pesos), dtype=np.float32)
    # xd[b, i, j, r] = x[b, 0, 4i+r, 4j+r]
    xp = x.reshape(B, GH, R, GW, R)
    xd = np.einsum("birjr->bijr", xp).reshape(B, N, R)
    # wd[n, ds, r] = pesos[n, d, s, r, r]
    wd = pesos.reshape(N, DS, R * R)[:, :, :: R + 1]  # [N, 512, 4]

    in_maps = []
    for k in range(NCORES):
        n0 = k * NPC
        xdk = xd[:, n0:n0 + NPC, :]   # [B, 128, 4]
        wdk = wd[n0:n0 + NPC]         # [128, 512, 4]
        # wt[c*4+r, p*512+ds] = wdk[2p+c, ds, r]
        wtk = np.ascontiguousarray(
            wdk.reshape(PAIRS, 2, DS, R).transpose(1, 3, 0, 2)
        ).reshape(8, PAIRS * DS)
        # xbd[c*4+r, p*128 + c*64 + b] = xdk[b, 2p+c, r] (block-diagonal)
        A = xdk.reshape(B, PAIRS, 2, R).transpose(2, 3, 1, 0)  # [c, r, p, b]
        L = np.zeros((2, R, PAIRS, 2, B), dtype=np.float32)
        L[0, :, :, 0, :] = A[0]
        L[1, :, :, 1, :] = A[1]
        xbdk = L.reshape(8, PAIRS * 128)
        in_maps.append({"xbd": xbdk, "wt": wtk})
    return in_maps


TRACE = {"on": False, "last": None}


def kernel(x, pesos):
    from concourse.bass_utils import run_bass_kernel_spmd

    in_maps = _pack_inputs(x, pesos)
    nc = _build_bass()
    res = run_bass_kernel_spmd(
        nc, in_maps, core_ids=list(range(NCORES)), trace=TRACE["on"]
    )
    TRACE["last"] = res
    outs = [res.results[k]["out"] for k in range(NCORES)]
    full = np.concatenate(outs, axis=1)  # [B, N, DS]
    return np.ascontiguousarray(full).reshape(B, N, D, S)


# revision 8
# speedup vs baseline: 1.1175x; 1.1175x over previous
"""Trainium2 Bass kernel for nn_ConexaoRegional.

Reference computation:
    out[b, n, d, s] = sum_r xd[b, n, r] * wd[n, d, s, r]
where
    xd[b, (i,j), r] = x[b, 0, 4i+r, 4j+r]     (patch diagonal)
    wd[n, d, s, r]  = pesos[n, d, s, r, r]    (weight diagonal)

Shapes: x [64,1,128,128] f32, pesos [1024,16,32,4,4] f32,
        out [64,1024,16,32] f32 (128 MiB -> memory-bound).

Strategy: shard the region axis (n) across 8 cores (128 regions each,
contiguous patch-row blocks). Host packs, per core:
  - a block-diagonal stationary operand "xbd" [8, 64*128]: for each pair
    of regions (2p, 2p+1), an [8, 128] tile whose rows are (c*4+r) and
    cols (c*64+b), holding xd values on the two diagonal blocks.
  - a moving operand "wt" [8, 64*512]: for each pair an [8, 512] tile
    with wt[c*4+r, ds] = wd[2p+c, ds, r].
On device each pair is one K=8 matmul -> PSUM [128, 512] = out for the
2 regions x 64 batches x 512 (d,s); copy PSUM->SBUF (alternating
Vector/Scalar engines) and stream 2 MiB chunks to DRAM with 2 KiB
contiguous runs. All FLOPs (the einsum contraction) happen on device.
"""

import numpy as np

B = 64
R = 4
GH = GW = 32
N = GH * GW            # 1024 regions
D, S = 16, 32
DS = D * S             # 512
NCORES = 8
NPC = N // NCORES      # 128 regions per core
PAIRS = NPC // 2       # 64 matmuls per core
GCH = 4                # pairs per output store chunk
NCHUNK = PAIRS // GCH  # 16 chunks -> 2 MiB per store DMA

_NC_CACHE = {}


def _build_bass():
    if "nc" in _NC_CACHE:
        return _NC_CACHE["nc"]
    from contextlib import ExitStack

    import concourse.bacc as bacc
    import concourse.mybir as mybir
    import concourse.tile as tile

    f32 = mybir.dt.float32
    f16 = mybir.dt.float16
    nc = bacc.Bacc()  # Bacc (not raw Bass): its compile passes split multi-sem
    # waits and move matmul waits to ldweights, which TRN2 codegen requires.
    # fp32 matmul runs ~8x slower than 16-bit on TRN2 (2 passes @ 1/4 rate), so
    # inputs arrive as fp16 hi/lo splits; 3-term accumulation in fp32 PSUM
    # (hi@hi + hi@lo + lo@hi) carries ~2^-22 relative error.
    xbd = nc.declare_dram_parameter("xbd", [8, 2 * PAIRS * 128], f16, isOutput=False)
    wt = nc.declare_dram_parameter("wt", [8, 2 * PAIRS * DS], f16, isOutput=False)
    out = nc.declare_dram_parameter("out", [B, NPC, DS], f32, isOutput=True)
    XL = PAIRS * 128  # offset of the lo half in xbd's free dim
    WL = PAIRS * DS   # offset of the lo half in wt's free dim

    with ExitStack() as ctx:
        tc = ctx.enter_context(tile.TileContext(nc))
        const = ctx.enter_context(tc.tile_pool(name="const", bufs=1))
        wpool = ctx.enter_context(tc.tile_pool(name="wtp", bufs=3))
        pspool = ctx.enter_context(tc.tile_pool(name="ps", bufs=6, space="PSUM"))
        opool = ctx.enter_context(tc.tile_pool(name="ostage", bufs=3))

        xbd_sb = const.tile([8, 2 * PAIRS * 128], f16)
        nc.sync.dma_start(xbd_sb[:], xbd[:])

        for g in range(NCHUNK):
            wt_sb = wpool.tile([8, 2 * GCH * DS], f16)
            nc.sync.dma_start(
                wt_sb[:, :GCH * DS], wt[:, g * GCH * DS:(g + 1) * GCH * DS]
            )
            nc.sync.dma_start(
                wt_sb[:, GCH * DS:], wt[:, WL + g * GCH * DS:WL + (g + 1) * GCH * DS]
            )
            ostage = opool.tile([128, GCH * DS], f32)
            for j in range(GCH):
                p = g * GCH + j
                xh = xbd_sb[:, p * 128:(p + 1) * 128]
                xl = xbd_sb[:, XL + p * 128:XL + (p + 1) * 128]
                wh = wt_sb[:, j * DS:(j + 1) * DS]
                wl = wt_sb[:, GCH * DS + j * DS:GCH * DS + (j + 1) * DS]
                ps = pspool.tile([128, DS], f32)
                nc.tensor.matmul(ps[:], lhsT=xh, rhs=wh, start=True, stop=False)
                nc.tensor.matmul(ps[:], lhsT=xh, rhs=wl, start=False, stop=False)
                nc.tensor.matmul(ps[:], lhsT=xl, rhs=wh, start=False, stop=True)
                dst = ostage[:, j * DS:(j + 1) * DS]
                if j % 2 == 0:
                    nc.vector.tensor_copy(dst, ps[:])
                else:
                    nc.scalar.copy(dst, ps[:])
            # out[b, 8g + 2p' + c, ds]; src rows are (c*64+b), free (p', ds).
            # DMA APs max 3 dims -> one store per parity c: dst [b, p', ds].
            for c in range(2):
                dst_ap = out[:, 8 * g + c:8 * (g + 1):2, :]
                nc.sync.dma_start(dst_ap, ostage[c * 64:(c + 1) * 64, :])

    nc.compile()  # Bacc passes: reg alloc, wait splitting, ldweights fixup
    _NC_CACHE["nc"] = nc
    return nc


def _pack_inputs(x, pesos):
    x = np.ascontiguousarray(np.asarray(x), dtype=np.float32)
    pesos = np.ascontiguousarray(np.asarray(pesos), dtype=np.float32)
    # xd[b, i, j, r] = x[b, 0, 4i+r, 4j+r]
    xp = x.reshape(B, GH, R, GW, R)
    xd = np.einsum("birjr->bijr", xp).reshape(B, N, R)
    # wd[n, ds, r] = pesos[n, d, s, r, r]
    wd = pesos.reshape(N, DS, R * R)[:, :, :: R + 1]  # [N, 512, 4]

    def hilo(a):
        hi = a.astype(np.float16)
        lo = (a - hi.astype(np.float32)).astype(np.float16)
        return hi, lo

    in_maps = []
    for k in range(NCORES):
        n0 = k * NPC
        xdk = xd[:, n0:n0 + NPC, :]   # [B, 128, 4]
        wdk = wd[n0:n0 + NPC]         # [128, 512, 4]
        # wt[c*4+r, p*512+ds] = wdk[2p+c, ds, r]
        wtk = np.ascontiguousarray(
            wdk.reshape(PAIRS, 2, DS, R).transpose(1, 3, 0, 2)
        ).reshape(8, PAIRS * DS)
        # xbd[c*4+r, p*128 + c*64 + b] = xdk[b, 2p+c, r] (block-diagonal)
        A = xdk.reshape(B, PAIRS, 2, R).transpose(2, 3, 1, 0)  # [c, r, p, b]
        L = np.zeros((2, R, PAIRS, 2, B), dtype=np.float32)
        L[0, :, :, 0, :] = A[0]
        L[1, :, :, 1, :] = A[1]
        xbdk = L.reshape(8, PAIRS * 128)
        xh, xl = hilo(xbdk)
        wh, wl = hilo(wtk)
        in_maps.append(
            {
                "xbd": np.concatenate([xh, xl], axis=1),
                "wt": np.concatenate([wh, wl], axis=1),
            }
        )
    return in_maps


TRACE = {"on": False, "last": None}


def kernel(x, pesos):
    from concourse.bass_utils import run_bass_kernel_spmd

    in_maps = _pack_inputs(x, pesos)
    nc = _build_bass()
    res = run_bass_kernel_spmd(
        nc, in_maps, core_ids=list(range(NCORES)), trace=TRACE["on"]
    )
    TRACE["last"] = res
    outs = [res.results[k]["out"] for k in range(NCORES)]
    full = np.concatenate(outs, axis=1)  # [B, N, DS]
    return np.ascontiguousarray(full).reshape(B, N, D, S)


# revision 12
# speedup vs baseline: 2.4674x; 2.2079x over previous
"""Trainium2 Bass kernel for nn_ConexaoRegional.

Reference computation:
    out[b, n, d, s] = sum_r xd[b, n, r] * wd[n, d, s, r]
where
    xd[b, (i,j), r] = x[b, 0, 4i+r, 4j+r]     (patch diagonal)
    wd[n, d, s, r]  = pesos[n, d, s, r, r]    (weight diagonal)

Shapes: x [64,1,128,128] f32, pesos [1024,16,32,4,4] f32,
        out [64,1024,16,32] f32 (128 MiB -> memory-bound).

Strategy: shard the region axis (n) across 8 cores (128 regions each,
contiguous patch-row blocks). Host packs, per core and per pair of
regions (2p, 2p+1), a block-diagonal stationary operand whose rows are
(term, c, r) and cols (c*64+b), and a moving operand [K, 512] with the
matching wd rows. fp32 matmul runs ~8x slower than bf16 on TRN2, so
operands are split hi/lo in bf16 and the four product terms
(hh, hl, lh, ll) are folded into the CONTRACTION dim: K = 32 rows =
[xh;xl;xh;xl] against [wh;wh;wl;wl]. bf16 products are exact in the
fp32 PSUM accumulation, so the result carries only the double-bf16
representation error (~8e-6 relative). One 213 ns matmul per pair ->
PSUM [128, 512] = 2 regions x 64 batch x 512 (d,s). PSUM->SBUF copies
alternate Vector/Scalar engines; stores go out as one fully-contiguous
2 MiB DMA per 8-pair chunk (the DRAM buffer is written in engine order
[chunk, c, b, pair, ds] and un-permuted on the host). All FLOPs (the
einsum contraction) happen on device.
"""

import numpy as np

B = 64
R = 4
GH = GW = 32
N = GH * GW            # 1024 regions
D, S = 16, 32
DS = D * S             # 512
NCORES = 8
NPC = N // NCORES      # 128 regions per core
PAIRS = NPC // 2       # 64 pair-matmuls per core
GCH = 8                # pairs per output store chunk
NCHUNK = PAIRS // GCH  # 8 chunks -> 2 MiB contiguous per store DMA
CHUNK_ELEMS = 2 * B * GCH * DS  # 524288 f32 per chunk

_NC_CACHE = {}


def _build_bass():
    if "nc" in _NC_CACHE:
        return _NC_CACHE["nc"]
    from contextlib import ExitStack

    import concourse.bacc as bacc
    import concourse.mybir as mybir
    import concourse.tile as tile

    f32 = mybir.dt.float32
    bf16 = mybir.dt.bfloat16
    nc = bacc.Bacc()  # Bacc (not raw Bass): its compile passes split multi-sem
    # waits and move matmul waits to ldweights, which TRN2 codegen requires.

    # K = 32 rows: 4 hi/lo term blocks of 8 rows (c*4+r).
    xbd = nc.declare_dram_parameter("xbd", [32, PAIRS * 128], bf16, isOutput=False)
    wt = nc.declare_dram_parameter("wt", [32, PAIRS * DS], bf16, isOutput=False)
    out = nc.declare_dram_parameter("out", [NCHUNK, CHUNK_ELEMS], f32, isOutput=True)

    with ExitStack() as ctx:
        tc = ctx.enter_context(tile.TileContext(nc))
        const = ctx.enter_context(tc.tile_pool(name="const", bufs=1))
        wpool = ctx.enter_context(tc.tile_pool(name="wtp", bufs=3))
        pspool = ctx.enter_context(tc.tile_pool(name="ps", bufs=8, space="PSUM"))
        opool = ctx.enter_context(tc.tile_pool(name="ostage", bufs=3))

        xsb = const.tile([32, PAIRS * 128], bf16)
        nc.sync.dma_start(xsb[:], xbd[:])

        for g in range(NCHUNK):
            wsb = wpool.tile([32, GCH * DS], bf16)
            nc.sync.dma_start(wsb[:], wt[:, g * GCH * DS:(g + 1) * GCH * DS])
            ostage = opool.tile([128, GCH * DS], f32)
            for j in range(GCH):
                p = g * GCH + j
                ps = pspool.tile([128, DS], f32)
                nc.tensor.matmul(
                    ps[:],
                    lhsT=xsb[:, p * 128:(p + 1) * 128],
                    rhs=wsb[:, j * DS:(j + 1) * DS],
                    start=True,
                    stop=True,
                )
                dst = ostage[:, j * DS:(j + 1) * DS]
                if j % 2 == 0:
                    nc.vector.tensor_copy(dst, ps[:])
                else:
                    nc.scalar.copy(dst, ps[:])
            # ostage rows (c*64+b), free (j, ds) -> out[g] is written in
            # exactly that iteration order, so the store is contiguous.
            nc.sync.dma_start(out[g], ostage[:])

    nc.compile()  # Bacc passes: reg alloc, wait splitting, ldweights fixup
    _NC_CACHE["nc"] = nc
    return nc


def _pack_inputs(x, pesos):
    import ml_dtypes

    bf16 = ml_dtypes.bfloat16
    x = np.ascontiguousarray(np.asarray(x), dtype=np.float32)
    pesos = np.ascontiguousarray(np.asarray(pesos), dtype=np.float32)
    # xd[b, i, j, r] = x[b, 0, 4i+r, 4j+r]
    xp = x.reshape(B, GH, R, GW, R)
    xd = np.einsum("birjr->bijr", xp).reshape(B, N, R)
    # wd[n, ds, r] = pesos[n, d, s, r, r]
    wd = pesos.reshape(N, DS, R * R)[:, :, :: R + 1]  # [N, 512, 4]

    def hilo(a):
        hi = a.astype(bf16)
        lo = (a - hi.astype(np.float32)).astype(bf16)
        return hi, lo

    in_maps = []
    for k in range(NCORES):
        n0 = k * NPC
        xdk = xd[:, n0:n0 + NPC, :]   # [B, 128, 4]
        wdk = wd[n0:n0 + NPC]         # [128, 512, 4]
        # wt[c*4+r, p*512+ds] = wdk[2p+c, ds, r]
        wtk = np.ascontiguousarray(
            wdk.reshape(PAIRS, 2, DS, R).transpose(1, 3, 0, 2)
        ).reshape(8, PAIRS * DS)
        # xbd[c*4+r, p*128 + c*64 + b] = xdk[b, 2p+c, r] (block-diagonal)
        A = xdk.reshape(B, PAIRS, 2, R).transpose(2, 3, 1, 0)  # [c, r, p, b]
        L = np.zeros((2, R, PAIRS, 2, B), dtype=np.float32)
        L[0, :, :, 0, :] = A[0]
        L[1, :, :, 1, :] = A[1]
        xbdk = L.reshape(8, PAIRS * 128)
        xh, xl = hilo(xbdk)
        wh, wl = hilo(wtk)
        in_maps.append(
            {
                "xbd": np.ascontiguousarray(np.concatenate([xh, xl, xh, xl], axis=0)),
                "wt": np.ascontiguousarray(np.concatenate([wh, wh, wl, wl], axis=0)),
            }
        )
    return in_maps


TRACE = {"on": False, "last": None}


def kernel(x, pesos):
    from concourse.bass_utils import run_bass_kernel_spmd

    in_maps = _pack_inputs(x, pesos)
    nc = _build_bass()
    res = run_bass_kernel_spmd(
        nc, in_maps, core_ids=list(range(NCORES)), trace=TRACE["on"]
    )
    TRACE["last"] = res
    outs = []
    for k in range(NCORES):
        # res[g, c, b, j, ds] = out[b, GCH*2*g + 2j + c, ds]
        r = res.results[k]["out"].reshape(NCHUNK, 2, B, GCH, DS)
        outs.append(r.transpose(2, 0, 3, 1, 4).reshape(B, NPC, DS))
    full = np.concatenate(outs, axis=1)  # [B, N, DS]
    return np.ascontiguousarray(full).reshape(B, N, D, S)
